# revision 5
# baseline (speedup 1.0000x reference)
"""BackWarp (dense_image_warp, bilinear+clamp) Trainium2 Bass kernel.

Sharding: pure data parallelism, 4 of 32 images per NeuronCore.

Per-core algorithm:
  1. Patch table per image in DRAM: entry (y, b) = rows {y, y+1} x cols
     [9b, 9b+10) x 3ch = 60 f32 padded to 64 f32 (256B = dma_gather granule).
     Any bilinear 2x2 patch is inside entry (iy, ix//9) at lane 3*(ix%9).
  2. DVE computes per-pixel iy, ix, ay, ax (exact floor via +2^23 round trick)
     and the int16 table index iy*71 + ix//9.
  3. One dma_gather 256B descriptor per pixel (~1.4 ns/desc measured).
  4. DVE extracts the 12 patch lanes (9-way compare + copy_predicated) and
     applies the bilinear lerp; strided DMA writes the output.

Pixel-to-lane layout is chosen so dma_gather's wrapped-16 index order and
wrapped-128 output order both line up with contiguous DMA transfers:
  global pixel P = (16k+q)*7200 + 900*w + u,  k,w in [0,8), q in [0,16), u in [0,900)
  meta/output layout: partition p' = 16*w+q, free (k, u)
  idx layout:         partition p = 16*k+q, free m = 900*w+u, permuted to
                      s = 8*u+w order then replicated to all 16-partition groups.
"""
import sys
sys.path.insert(0, '/opt/trn_rl_repo')
import numpy as np

import concourse.bass as bass
import concourse.tile as tile
from concourse import bacc, mybir
from concourse.bass_utils import run_bass_kernel_spmd

F32 = mybir.dt.float32
I32 = mybir.dt.int32
I16 = mybir.dt.int16
I8 = mybir.dt.int8
ADD = mybir.AluOpType.add
SUB = mybir.AluOpType.subtract
MUL = mybir.AluOpType.mult
MAX_ = mybir.AluOpType.max
MIN_ = mybir.AluOpType.min
GT = mybir.AluOpType.is_gt
EQ = mybir.AluOpType.is_equal

B, H, W, C = 32, 360, 640, 3
NCORE = 8
BL = B // NCORE                 # 4 images per core
NPIX = BL * H * W               # 921600
FP = NPIX // 128                # 7200 pixels per partition
UC = FP // 8                    # 900
NBLK = 71                       # 9*71+1 = 640 column blocks per row
NTAB = (H - 1) * NBLK           # 25489 entries per image
ELEM = 64                       # 256B entries
UCH = 75                        # u per gather chunk
NCHUNK = UC // UCH              # 12
MAGIC = float(1.5 * 2.0 ** 23)

_CACHE = {}


def _ap(t, extra_offset, dims):
    return bass.AP(t.tensor, t.offset + extra_offset, [list(d) for d in dims])


def build_nc():
    nc = bacc.Bacc("TRN2", target_bir_lowering=False, debug=False,
                   num_devices=1, num_swdge_queues=2)
    frame_d = nc.dram_tensor("frame", [BL * H * W, C], F32, kind="ExternalInput")
    flow_d = nc.dram_tensor("flow", [BL * H * W, 2], F32, kind="ExternalInput")
    out_d = nc.dram_tensor("out", [BL * H * W, C], F32, kind="ExternalOutput")

    with tile.TileContext(nc) as tc:
        with tc.tile_pool(name="rtab", bufs=1, space="DRAM") as rpool:
            rtabs = [rpool.tile([NTAB, ELEM], F32, name=f'rtab{i}') for i in range(BL)]

            # ---------------- phase 1: patch tables --------------------------
            with tc.tile_pool(name="bld", bufs=2) as bld:
                for img in range(BL):
                    rt = rtabs[img]
                    for rb in (0, 128, 256):
                        L = min(128, (H - 1) - rb)
                        ta = bld.tile([128, W * C], F32, tag="ta")
                        tb = bld.tile([128, W * C], F32, tag="tb")
                        base = img * H * W * C
                        nc.sync.dma_start(
                            ta[:L, :], _ap(frame_d[:], base + rb * W * C,
                                           [[W * C, L], [1, W * C]]))
                        nc.sync.dma_start(
                            tb[:L, :], _ap(frame_d[:], base + (rb + 1) * W * C,
                                           [[W * C, L], [1, W * C]]))
                        rtile = bld.tile([128, NBLK * ELEM], F32, tag="rt")
                        ps = rtile[:].ap[0][0]
                        pa = ta[:].ap[0][0]
                        nc.vector.tensor_copy(
                            _ap(rtile[:L], 0, [[ps, L], [ELEM, NBLK], [1, 30]]),
                            _ap(ta[:L], 0, [[pa, L], [27, NBLK], [1, 30]]))
                        nc.vector.tensor_copy(
                            _ap(rtile[:L], 30, [[ps, L], [ELEM, NBLK], [1, 30]]),
                            _ap(tb[:L], 0, [[pa, L], [27, NBLK], [1, 30]]))
                        # column-major table: entry (bx, ey) at
                        # (bx*359 + ey)*ELEM so gather streams walk narrow
                        # column bands (HBM row-buffer locality)
                        nc.sync.dma_start(
                            _ap(rt[:], rb * ELEM,
                                [[ELEM, L], [(H - 1) * ELEM, NBLK],
                                 [1, ELEM]]),
                            rtile[:L, :])

            # ---------------- phase 2: main ----------------------------------
            with (
                tc.tile_pool(name="cst", bufs=1) as cst,
                tc.tile_pool(name="meta", bufs=1) as meta,
                tc.tile_pool(name="idxp", bufs=1) as idxp,
                tc.tile_pool(name="c2rp", bufs=2) as c2rp,
                tc.tile_pool(name="gath", bufs=2) as gath,
            ):
                # per-partition constants
                pidx = cst.tile([128, 1], I32)
                nc.gpsimd.iota(pidx[:], pattern=[[0, 1]], base=0,
                               channel_multiplier=1)
                pf = cst.tile([128, 1], F32)
                nc.vector.tensor_copy(pf[:], pidx[:])
                blk = cst.tile([128, 1], F32, tag="blk")
                nc.vector.tensor_scalar(blk[:], pf[:], float(1.0 / 16.0),
                                        float(-0.46875), op0=MUL, op1=ADD)
                nc.vector.tensor_scalar(blk[:], blk[:], MAGIC, -MAGIC,
                                        op0=ADD, op1=ADD)       # p//16
                qf = cst.tile([128, 1], F32, tag="qf")
                nc.vector.tensor_scalar(qf[:], blk[:], float(-16.0), None,
                                        op0=MUL)
                nc.vector.tensor_tensor(qf[:], qf[:], pf[:], op=ADD)  # p%16
                basem = cst.tile([128, 1], F32, tag="basem")    # q*7200 + w*900
                nc.vector.tensor_scalar(basem[:], qf[:], float(7200.0), None,
                                        op0=MUL)
                t1 = cst.tile([128, 1], F32, tag="t1")
                nc.vector.tensor_scalar(t1[:], blk[:], float(900.0), None,
                                        op0=MUL)
                nc.vector.tensor_tensor(basem[:], basem[:], t1[:], op=ADD)
                # idx layout: local base = q*7200 + 115200*(k%2), p=16k+q
                km2 = cst.tile([128, 1], F32, tag="km2")
                nc.vector.tensor_scalar(km2[:], blk[:], float(0.5),
                                        float(-0.25), op0=MUL, op1=ADD)
                nc.vector.tensor_scalar(km2[:], km2[:], MAGIC, -MAGIC,
                                        op0=ADD, op1=ADD)       # k//2
                nc.vector.tensor_scalar(km2[:], km2[:], float(-2.0), None,
                                        op0=MUL)
                nc.vector.tensor_tensor(km2[:], km2[:], blk[:], op=ADD)  # k%2
                basei = cst.tile([128, 1], F32, tag="basei")
                nc.vector.tensor_scalar(basei[:], qf[:], float(7200.0), None,
                                        op0=MUL)
                nc.vector.tensor_scalar(t1[:], km2[:], float(115200.0), None,
                                        op0=MUL)
                nc.vector.tensor_tensor(basei[:], basei[:], t1[:], op=ADD)

                iota_u = cst.tile([128, UC], I32)
                nc.gpsimd.iota(iota_u[:], pattern=[[1, UC]], base=0,
                               channel_multiplier=0)
                iota_uf = cst.tile([128, UC], F32)
                nc.vector.tensor_copy(iota_uf[:], iota_u[:])

                def mtile(tag):
                    return meta.tile([128, UC], F32, tag=tag, name=tag)

                def floorx(dst, src, tmp):
                    nc.vector.tensor_scalar(dst[:], src[:], MAGIC, -MAGIC,
                                            op0=ADD, op1=ADD)
                    nc.vector.tensor_tensor(tmp[:], dst[:], src[:], op=GT)
                    nc.vector.tensor_tensor(dst[:], dst[:], tmp[:], op=SUB)

                def clip(dst, lo, hi):
                    nc.vector.tensor_scalar(dst[:], dst[:], float(lo),
                                            float(hi), op0=MAX_, op1=MIN_)

                def yx_from_loc(loc, yv, xv):
                    nc.vector.tensor_scalar(yv[:], loc[:], float(-319.5),
                                            None, op0=ADD)
                    nc.vector.tensor_scalar(yv[:], yv[:], float(1.0 / 640.0),
                                            MAGIC, op0=MUL, op1=ADD)
                    nc.vector.tensor_scalar(yv[:], yv[:], -MAGIC, None, op0=ADD)
                    nc.vector.tensor_scalar(xv[:], yv[:], float(-640.0),
                                            None, op0=MUL)
                    nc.vector.tensor_tensor(xv[:], xv[:], loc[:], op=ADD)

                # ---- idx pass: c2[p, 900w+u] = int16 table index ------------
                c2 = idxp.tile([128, FP], I16)
                for mb in range(8):
                    fl = meta.tile([128, UC * 2], F32, tag="fl")
                    nc.sync.dma_start(
                        fl[:], _ap(flow_d[:], mb * UC * 2,
                                   [[FP * 2, 128], [1, UC * 2]]))
                    fy, fx = mtile("fy"), mtile("fx")
                    st = fl[:].ap[0][0]
                    nc.vector.tensor_copy(fy[:], _ap(fl[:], 0, [[st, 128], [2, UC]]))
                    nc.vector.tensor_copy(fx[:], _ap(fl[:], 1, [[st, 128], [2, UC]]))
                    loc = mtile("loc")
                    nc.vector.tensor_scalar(loc[:], iota_uf[:],
                                            float(mb * UC), basei[:],
                                            op0=ADD, op1=ADD)
                    yv, xv = mtile("yv"), mtile("xv")
                    yx_from_loc(loc, yv, xv)
                    qv, iv, tmp = mtile("qv"), mtile("iv"), mtile("tmp")
                    nc.vector.tensor_tensor(qv[:], yv[:], fy[:], op=SUB)
                    floorx(iv, qv, tmp)
                    clip(iv, 0.0, float(H - 2))
                    idxv = mtile("idxv")
                    nc.vector.tensor_copy(idxv[:], iv[:])
                    nc.vector.tensor_tensor(qv[:], xv[:], fx[:], op=SUB)
                    floorx(iv, qv, tmp)
                    clip(iv, 0.0, float(W - 2))
                    bv = mtile("bv")
                    nc.vector.tensor_scalar(bv[:], iv[:], float(-4.0), None,
                                            op0=ADD)
                    nc.vector.tensor_scalar(bv[:], bv[:], float(1.0 / 9.0),
                                            MAGIC, op0=MUL, op1=ADD)
                    nc.vector.tensor_scalar(bv[:], bv[:], -MAGIC, None, op0=ADD)
                    nc.vector.tensor_scalar(bv[:], bv[:], float(H - 1), None,
                                            op0=MUL)
                    nc.vector.tensor_tensor(idxv[:], idxv[:], bv[:], op=ADD)
                    nc.vector.tensor_copy(c2[:, mb * UC:(mb + 1) * UC], idxv[:])
                c2p = idxp.tile([128, FP], I16)
                nc.vector.tensor_copy(
                    c2p[:], _ap(c2[:], 0,
                                [[c2[:].ap[0][0], 128], [1, UC], [UC, 8]]))

                # ---- slabs ---------------------------------------------------
                for k in range(8):
                    img = k // 2
                    c2r = c2rp.tile([128, FP], I16, tag="c2r")
                    for g8 in range(8):
                        nc.sync.dma_start(c2r[16 * g8:16 * g8 + 16, :],
                                          c2p[16 * k:16 * k + 16, :])
                    flm = meta.tile([128, UC * 2], F32, tag="fl")
                    nc.sync.dma_start(
                        flm[:], _ap(flow_d[:], k * 16 * FP * 2,
                                    [[UC * 2, 8], [FP * 2, 16], [1, UC * 2]]))
                    fy, fx = mtile("fy"), mtile("fx")
                    st = flm[:].ap[0][0]
                    nc.vector.tensor_copy(fy[:], _ap(flm[:], 0, [[st, 128], [2, UC]]))
                    nc.vector.tensor_copy(fx[:], _ap(flm[:], 1, [[st, 128], [2, UC]]))
                    loc = mtile("loc")
                    nc.vector.tensor_scalar(loc[:], iota_uf[:],
                                            float(115200.0 * (k % 2)),
                                            basem[:], op0=ADD, op1=ADD)
                    yv, xv = mtile("yv"), mtile("xv")
                    yx_from_loc(loc, yv, xv)
                    qv, tmp = mtile("qv"), mtile("tmp")
                    iy, ay = mtile("iy"), mtile("ay")
                    nc.vector.tensor_tensor(qv[:], yv[:], fy[:], op=SUB)
                    floorx(iy, qv, tmp)
                    clip(iy, 0.0, float(H - 2))
                    nc.vector.tensor_tensor(ay[:], qv[:], iy[:], op=SUB)
                    clip(ay, 0.0, 1.0)
                    ix, ax = mtile("ix"), mtile("ax")
                    nc.vector.tensor_tensor(qv[:], xv[:], fx[:], op=SUB)
                    floorx(ix, qv, tmp)
                    clip(ix, 0.0, float(W - 2))
                    nc.vector.tensor_tensor(ax[:], qv[:], ix[:], op=SUB)
                    clip(ax, 0.0, 1.0)
                    bv, c0 = mtile("bv"), mtile("c0")
                    nc.vector.tensor_scalar(bv[:], ix[:], float(-4.0), None,
                                            op0=ADD)
                    nc.vector.tensor_scalar(bv[:], bv[:], float(1.0 / 9.0),
                                            MAGIC, op0=MUL, op1=ADD)
                    nc.vector.tensor_scalar(bv[:], bv[:], -MAGIC, None, op0=ADD)
                    nc.vector.tensor_scalar(c0[:], bv[:], float(-9.0), None,
                                            op0=MUL)
                    nc.vector.tensor_tensor(c0[:], c0[:], ix[:], op=ADD)

                    for j in range(NCHUNK):
                        g = gath.tile([128, UCH * ELEM], F32, tag="g")
                        nc.gpsimd.dma_gather(
                            out_ap=g[:].rearrange("p (n e) -> p n e", e=ELEM),
                            in_ap=rtabs[img][:],
                            idxs_ap=c2r[:, j * UCH * 8:(j + 1) * UCH * 8],
                            num_idxs=UCH * 128,
                            num_idxs_reg=UCH * 128,
                            elem_size=ELEM,
                            single_packet=False,
                            queue_num=(k * NCHUNK + j) % 2)
                        e = gath.tile([128, UCH * 12], F32, tag="e")
                        gst = g[:].ap[0][0]
                        est = e[:].ap[0][0]
                        msk = gath.tile([128, UCH], I8, tag="msk")
                        mst = msk[:].ap[0][0]
                        c0j = c0[:, j * UCH:(j + 1) * UCH]
                        for k0 in range(9):
                            nc.vector.tensor_scalar(msk[:], c0j, float(k0),
                                                    None, op0=EQ)
                            nc.vector.copy_predicated(
                                _ap(e[:], 0, [[est, 128], [12, UCH], [6, 2], [1, 6]]),
                                _ap(msk[:], 0, [[mst, 128], [1, UCH], [0, 2], [0, 6]]),
                                _ap(g[:], 3 * k0, [[gst, 128], [ELEM, UCH],
                                                   [30, 2], [1, 6]]))
                        axj = ax[:, j * UCH:(j + 1) * UCH]
                        ayj = ay[:, j * UCH:(j + 1) * UCH]
                        ast = ax[:].ap[0][0]
                        d = gath.tile([128, UCH * 6], F32, tag="d")
                        nc.vector.tensor_tensor(
                            d[:],
                            _ap(e[:], 3, [[est, 128], [12, UCH], [6, 2], [1, 3]]),
                            _ap(e[:], 0, [[est, 128], [12, UCH], [6, 2], [1, 3]]),
                            op=SUB)
                        nc.vector.tensor_tensor(
                            d[:], d[:],
                            _ap(axj, 0, [[ast, 128], [1, UCH], [0, 2], [0, 3]]),
                            op=MUL)
                        tb2 = gath.tile([128, UCH * 6], F32, tag="tb2")
                        nc.vector.tensor_tensor(
                            tb2[:], d[:],
                            _ap(e[:], 0, [[est, 128], [12, UCH], [6, 2], [1, 3]]),
                            op=ADD)
                        t2s = tb2[:].ap[0][0]
                        o = gath.tile([128, UCH * 3], F32, tag="o")
                        nc.vector.tensor_tensor(
                            o[:],
                            _ap(tb2[:], 3, [[t2s, 128], [6, UCH], [1, 3]]),
                            _ap(tb2[:], 0, [[t2s, 128], [6, UCH], [1, 3]]),
                            op=SUB)
                        nc.vector.tensor_tensor(
                            o[:], o[:],
                            _ap(ayj, 0, [[ast, 128], [1, UCH], [0, 3]]),
                            op=MUL)
                        nc.vector.tensor_tensor(
                            o[:], o[:],
                            _ap(tb2[:], 0, [[t2s, 128], [6, UCH], [1, 3]]),
                            op=ADD)
                        nc.sync.dma_start(
                            _ap(out_d[:], (k * 16 * FP + j * UCH) * 3,
                                [[UC * 3, 8], [FP * 3, 16], [1, UCH * 3]]),
                            o[:])
    nc.finalize()
    return nc


def kernel(frame_tail, flow):
    frame_tail = np.ascontiguousarray(frame_tail, dtype=np.float32)
    flow = np.ascontiguousarray(flow, dtype=np.float32)
    if "nc" not in _CACHE:
        _CACHE["nc"] = build_nc()
    nc = _CACHE["nc"]
    in_maps = []
    for c in range(NCORE):
        fr = frame_tail[c * BL:(c + 1) * BL].reshape(BL * H * W, C)
        fl = flow[c * BL:(c + 1) * BL].reshape(BL * H * W, 2)
        in_maps.append({"frame": fr, "flow": fl})
    res = run_bass_kernel_spmd(nc, in_maps, core_ids=list(range(NCORE)))
    out = np.empty((B, H, W, C), np.float32)
    for c in range(NCORE):
        out[c * BL:(c + 1) * BL] = res.results[c]["out"].reshape(BL, H, W, C)
    return out

